# revision 11
# baseline (speedup 1.0000x reference)
"""Trainium2 Bass kernel for 16-head MHA (E=1024, S=2048, B=4) on 8 NeuronCores.

Sharding: tensor-parallel over head groups (TP=2: heads 0-7 / 8-15) x
data-parallel over batch (DP=4).  Core c handles batch c//2, head group c%2.
Each core computes its 8 heads end-to-end plus the out-projection restricted
to its heads' rows of W_out; the host sums the two TP partials and adds b_out.

Device-side dataflow per core (matmuls in float32r at full PE rate):
  phase V : V[s, h*64+d] = x @ Wv + bv          (bias via K=1 augmented matmul)
  phase QK: QK^T[m, s]   = [Wq*0.125 | Wk]^T chunks @ x^T   (bias fused in the
            PSUM->SBUF copy; attention scale pre-folded into Wq/bq on host)
  attn    : scores^T[t, s] per head (2 heads packed in the 128-wide PE array
            via tile_position), exp on ACT over 1024-wide PSUM APs, then
            o_aug[65, s] = [V | 1]^T @ exp_scores^T giving o_unnorm^T and the
            softmax denominator in one accumulation; normalize with
            reciprocal + gpsimd partition_broadcast + DVE multiply.
  out     : out[s, e] = sum_k O^T[k-chunk, s-tile]^T @ Wo[k-chunk, e]
"""

import numpy as np

import concourse.bass as bass
import concourse.tile as tile
from concourse import bacc, mybir
from concourse.alu_op_type import AluOpType
from concourse.bass_utils import run_bass_kernel_spmd

F32 = mybir.dt.float32
F32R = mybir.dt.float32r
MM_DT = F32R      # matmul operand dtype: F32R (fast) or F32 (exact, 4x slower)
EXP = mybir.ActivationFunctionType.Exp

E = 1024          # embed dim
S = 2048          # sequence
B = 4             # batch
NH = 16           # total heads
HD = 64           # head dim
TP = 2            # head-group shards
HPC = NH // TP    # heads per core = 8
QKW = HPC * HD * 2   # 1024 q+k columns per core
VW = HPC * HD        # 512 v columns per core

KCH = E // 128      # 8 contraction chunks
MT = QKW // 128     # 8 qk^T row tiles (0-3 Q, 4-7 K)
ST = S // 128       # 16 sequence tiles
SC = S // 512       # 4 sequence 512-chunks

_CACHE = {}


def _mm(ap):
    """DRAM-side view matching the MM operand dtype (byte-identical)."""
    return ap.bitcast(MM_DT) if MM_DT is not F32 else ap


def build_nc():
    nc = bacc.Bacc("TRN2", target_bir_lowering=False, debug=False, num_devices=8)

    xT_d = nc.dram_tensor("xT", [E, S], F32, kind="ExternalInput").ap()
    wqk_d = nc.dram_tensor("wqk", [E, QKW], F32, kind="ExternalInput").ap()
    bqk_d = nc.dram_tensor("bqk", [128, MT], F32, kind="ExternalInput").ap()
    wv_d = nc.dram_tensor("wv", [E, VW], F32, kind="ExternalInput").ap()
    bv_d = nc.dram_tensor("bv", [1, VW], F32, kind="ExternalInput").ap()
    wo_d = nc.dram_tensor("wo", [VW, E], F32, kind="ExternalInput").ap()
    out_d = nc.dram_tensor("out", [S, E], F32, kind="ExternalOutput").ap()

    xT_t = xT_d.rearrange("(k p) s -> k p s", p=128)
    wv_t = wv_d.rearrange("(k p) c -> k p c", p=128)
    wo_t = wo_d.rearrange("(k p) c -> k p c", p=128)

    with tile.TileContext(nc) as tc:
        with tc.tile_pool(name="persist", bufs=1) as pp:
            bqk_sb = pp.tile([128, MT], F32, tag="bqk")
            nc.sync.dma_start(bqk_sb[:], bqk_d[:])
            bv_sb = pp.tile([1, VW], MM_DT, tag="bv")
            nc.sync.dma_start(bv_sb[:], _mm(bv_d[:]))
            ones_f32 = pp.tile([128, 128], F32, tag="ones_f32")
            nc.vector.memset(ones_f32[:], 1.0)
            ones_sb = pp.tile([1, 128], MM_DT, tag="ones")
            nc.vector.tensor_copy(ones_sb[:], ones_f32[0:1, :])

            qkt = [pp.tile([128, S], MM_DT, tag=f"qkt{m}", name=f"qkt{m}")
                   for m in range(MT)]
            vaug = [pp.tile([128, HPC * 65], MM_DT, tag=f"va{st}", name=f"va{st}")
                    for st in range(ST)]

            # ---------------- phase 1: load x^T and weights ----------------
            # (loads issued from the ACT queue, idle during this phase)
            with tc.tile_pool(name="p1", bufs=1) as p1:
                xt = [p1.tile([128, S], MM_DT, tag=f"xt{k}", name=f"xt{k}")
                      for k in range(KCH)]
                for k in range(KCH):
                    nc.scalar.dma_start(xt[k][:], _mm(xT_t[k]))
                wv_sb = [p1.tile([128, VW], MM_DT, tag=f"wv{k}", name=f"wv{k}")
                         for k in range(KCH)]
                for k in range(KCH):
                    nc.scalar.dma_start(wv_sb[k][:], _mm(wv_t[k]))

                # ---------- phase V: V in natural [s, h*64+d] layout ----------
                with tc.tile_pool(name="vps", bufs=4, space="PSUM") as vps:
                    for st in range(ST):
                        vp = vps.tile([128, VW], F32, tag="vp")
                        for k in range(KCH):
                            nc.tensor.matmul(
                                vp[:], xt[k][:, st * 128:(st + 1) * 128],
                                wv_sb[k][:], start=(k == 0), stop=False)
                        nc.tensor.matmul(
                            vp[:], ones_sb[:, :128], bv_sb[:],
                            start=False, stop=True)
                        va = vaug[st].rearrange("p (h c) -> p h c", c=65)
                        nc.vector.tensor_copy(va[:, :, 64:65], ones_f32[:, 0:8])
                        nc.vector.tensor_copy(
                            va[:, :, 0:64], vp[:].rearrange("p (h d) -> p h d", d=64))

                # ---------- phase QK: QK^T = W^T @ x^T with fused bias ----------
                with (tc.tile_pool(name="qkps", bufs=8, space="PSUM") as qkps,
                      tc.tile_pool(name="wqkp", bufs=2) as wqkp):
                    for m in range(MT):
                        # whole [E, 128] weight column-block in one DMA,
                        # laid out [128, k*128+c]
                        wm = wqkp.tile([128, E], MM_DT, tag="wm")
                        nc.scalar.dma_start(
                            wm[:].rearrange("p (k c) -> p k c", c=128),
                            _mm(wqk_d[:, m * 128:(m + 1) * 128].rearrange(
                                "(k p) c -> p k c", p=128)))
                        pss = [qkps.tile([128, 512], F32, tag="qk", name=f"qkps{m}_{j}")
                               for j in range(SC)]
                        for k in range(KCH):
                            for j in range(SC):
                                nc.tensor.matmul(
                                    pss[j][:], wm[:, k * 128:(k + 1) * 128],
                                    xt[k][:, j * 512:(j + 1) * 512],
                                    start=(k == 0), stop=(k == KCH - 1))
                        for j in range(SC):
                            nc.vector.tensor_scalar_add(
                                qkt[m][:, j * 512:(j + 1) * 512], pss[j][:],
                                bqk_sb[:, m:m + 1])

            # ---------- phase 2+3: attention, out-proj overlapped ----------
            with (tc.tile_pool(name="otp", bufs=1) as otp,
                  tc.tile_pool(name="scps", bufs=2, space="PSUM") as scps,
                  tc.tile_pool(name="oaps", bufs=2, space="PSUM") as oaps,
                  tc.tile_pool(name="ops", bufs=1, space="PSUM") as ops,
                  tc.tile_pool(name="attn", bufs=3) as ap_,
                  tc.tile_pool(name="wop", bufs=1) as wop,
                  tc.tile_pool(name="obp", bufs=3) as obp):
                ot = [otp.tile([128, S], MM_DT, tag=f"ot{hp}", name=f"ot{hp}")
                      for hp in range(TP * 2)]
                wo_sb = [wop.tile([128, E], MM_DT, tag=f"wo{k}", name=f"wo{k}")
                         for k in range(4)]
                for k in range(4):
                    nc.sync.dma_start(wo_sb[k][:], _mm(wo_t[k]))
                attention_phase(nc, tc, scps, oaps, ap_, qkt, vaug, ot)
                for st in range(ST):
                    op = ops.tile([128, E], F32, tag="op")
                    for e in range(2):
                        for k in range(4):
                            nc.tensor.matmul(
                                op[:, e * 512:(e + 1) * 512],
                                ot[k][:, st * 128:(st + 1) * 128],
                                wo_sb[k][:, e * 512:(e + 1) * 512],
                                start=(k == 0), stop=(k == 3))
                    ob = obp.tile([128, E], F32, tag="ob")
                    nc.vector.tensor_copy(ob[:], op[:])
                    nc.sync.dma_start(out_d[st * 128:(st + 1) * 128, :], ob[:])

    nc.compile()
    return nc


def attention_phase(nc, tc, scps, oaps, ap_, qkt, vaug, ot):
    for hp in range(HPC // 2):
        qt, kt = qkt[hp], qkt[4 + hp]
        for j in range(SC):
            oa0 = oaps.tile([65, 512], F32, tag="oa")
            oa1 = oaps.tile([65, 512], F32, tag="oa")
            for t in range(ST):
                sc = scps.tile([128, 1024], F32, tag="sc")
                et = ap_.tile([128, 1024], MM_DT, tag="et")
                for h in range(2):
                    nc.tensor.matmul(
                        sc[:, h * 512:(h + 1) * 512],
                        kt[h * 64:(h + 1) * 64, t * 128:(t + 1) * 128],
                        qt[h * 64:(h + 1) * 64, j * 512:(j + 1) * 512],
                        start=True, stop=True,
                        tile_position=(h * 64, 0))
                nc.scalar.activation(et[:], sc[:], EXP)
                for h, oa in ((0, oa0), (1, oa1)):
                    hh = hp * 2 + h
                    nc.tensor.matmul(
                        oa[:], vaug[t][:, hh * 65:(hh + 1) * 65],
                        et[:, h * 512:(h + 1) * 512],
                        start=(t == 0), stop=(t == ST - 1))
            for h, oa in ((0, oa0), (1, oa1)):
                recip = ap_.tile([1, 512], F32, tag="recip")
                nc.vector.reciprocal(recip[:], oa[64:65, :])
                rb = ap_.tile([64, 512], F32, tag="rb")
                nc.gpsimd.partition_broadcast(rb[:], recip[:])
                nc.vector.tensor_tensor(
                    ot[hp][h * 64:(h + 1) * 64, j * 512:(j + 1) * 512],
                    oa[0:64, :], rb[:], op=AluOpType.mult)


def _shard_inputs(x, W_qkv, b_qkv, W_out, b_out):
    """Build the 8 per-core input maps. Attention scale folded into Wq/bq."""
    scale = np.float32(HD ** -0.5)
    in_maps = []
    for c in range(8):
        b, g = c // TP, c % TP
        lo, hi = g * VW, (g + 1) * VW
        wq = W_qkv[:, lo:hi] * scale
        wk = W_qkv[:, E + lo:E + hi]
        wv = W_qkv[:, 2 * E + lo:2 * E + hi]
        bq = b_qkv[lo:hi] * scale
        bk = b_qkv[E + lo:E + hi]
        bv = b_qkv[2 * E + lo:2 * E + hi]
        bqk = np.concatenate([bq, bk]).reshape(MT, 128).T
        in_maps.append({
            "xT": np.ascontiguousarray(x[b].T),
            "wqk": np.ascontiguousarray(np.concatenate([wq, wk], axis=1)),
            "bqk": np.ascontiguousarray(bqk),
            "wv": np.ascontiguousarray(wv),
            "bv": np.ascontiguousarray(bv[None, :]),
            "wo": np.ascontiguousarray(W_out[lo:hi, :]),
        })
    return in_maps


def kernel(x, W_qkv, b_qkv, W_out, b_out):
    if "nc" not in _CACHE:
        _CACHE["nc"] = build_nc()
    nc = _CACHE["nc"]
    in_maps = _shard_inputs(x, W_qkv, b_qkv, W_out, b_out)
    res = run_bass_kernel_spmd(nc, in_maps, core_ids=list(range(8)))
    _CACHE["last_results"] = res
    out = np.empty((B, S, E), dtype=np.float32)
    for b in range(B):
        out[b] = res.results[TP * b]["out"] + res.results[TP * b + 1]["out"] + b_out
    return out
